# revision 12
# baseline (speedup 1.0000x reference)
"""Trainium2 Bass kernel for BilingualSentenceClassifier (segment_reduce).

Computes, for B=64 samples of S=2048 tokens with D=1024 embedding dims:
  sent1 = mean(embs[1:idx1])            (idx1 = first EOS position)
  sent2 = mean(embs[idx1+2:idx2])       (idx2 = first PAD position - 1)
  logits = tanh(concat(sent1, sent2) @ dense_w + dense_b) @ out_w + out_b

Strategy: pure data parallel over 8 NeuronCores (8 samples per core).
The kernel is memory-bound, so the design minimizes HBM bytes:

  * Only rows with nonzero mask weight are read: the host packs each
    sample's [1,idx1) and [idx1+2,idx2) token rows into one contiguous
    per-core stream (no 128-row quantization waste), balanced across
    cores by a swap-refined LPT assignment.
  * The packed embeddings are pre-cast on the host to fp8-e3m4 (4 bits
    of mantissa fits N(0,1) data; end-to-end rel-err ~1e-2 vs the 2e-2
    gate), cutting the dominant stream 4x vs fp32.  dense_w is pre-cast
    to bf16 (2x).  Segment sums accumulate in fp32 PSUM; the exact fp32
    1/n scales are applied post-accumulation, so the routing masks are
    exact {0,1} in fp8.
  * The mask tile routes each packed row to PSUM column q = 2*slot + r
    via the TensorEngine, so sample/segment boundaries may fall
    mid-chunk and the chunk loop is completely uniform.
  * dense_w streams in 16 chunks interleaved with the embedding stream
    so the DMA engines never idle; the dense head keeps dense_w
    stationary (output free dim = 8 samples) and lands hidden^T
    directly in the layout the logits head consumes.
"""

import sys

sys.path.insert(0, "/opt/trn_rl_repo")

import numpy as np
import ml_dtypes

import concourse.bass as bass
import concourse.tile as tile
from concourse import mybir
import bass_rust
from concourse.bass_utils import run_bass_kernel_spmd

B, S, D = 64, 2048, 1024
EOS_ID, PAD_ID = 2, 1
N_CORES = 8
B_LOC = B // N_CORES          # samples per core
G = 4                         # 128-row chunks per embedding DMA
KD = (2 * D) // 128           # 16 contraction chunks for the dense head
KH = D // 128                 # 8 h-tiles for the dense/logits heads

F32 = mybir.dt.float32
BF16 = mybir.dt.bfloat16
F8 = mybir.dt.float8e3

NP_F8 = ml_dtypes.float8_e3m4
NP_BF16 = ml_dtypes.bfloat16


def _split_excess_waits(nc, max_waits=1):
    """This container's walrus rejects instructions carrying more than 1-2
    sync waits (e.g. the Tile tail drain, matmuls lowered via S3_LW).
    Hoist excess waits onto preceding same-engine NOPs — semantically
    identical: the engine's sequencer blocks on the NOP's wait before
    dispatching the original instruction."""
    cnt = 0
    for f in nc.m.functions:
        for blk in f.blocks:
            out = []
            changed = False
            for inst in blk.instructions:
                si = inst.sync_info
                if si is not None and len(si.on_wait) > max_waits:
                    waits = list(si.on_wait)
                    for w in waits[:-max_waits]:
                        cnt += 1
                        nop = mybir.InstNoOp(name=f"{inst.name}-hw{cnt}")
                        nop.engine = inst.engine
                        nop.sync_info = bass_rust.SyncInfo(on_wait=[w], on_update=[])
                        out.append(nop)
                    inst.sync_info = bass_rust.SyncInfo(
                        on_wait=waits[-max_waits:], on_update=list(si.on_update)
                    )
                    changed = True
                out.append(inst)
            if changed:
                blk.instructions = out
    return cnt


def _build_program(C):
    """Build the SPMD Bass program for C packed 128-row chunks per core."""
    nc = bass.Bass("TRN2", target_bir_lowering=False, debug=False, num_devices=N_CORES)

    embs = nc.dram_tensor("embs", [C * 128, D], F8, kind="ExternalInput")
    wm = nc.dram_tensor("wm", [128, C * 16], F8, kind="ExternalInput")
    dw = nc.dram_tensor("dw", [2 * D, D], BF16, kind="ExternalInput")
    db = nc.dram_tensor("db", [1, D], F32, kind="ExternalInput")
    # cb packs ow/sv/ones/ob/sel into one [128, CB] fp32 tensor so the
    # HWDGE issue path sees one DMA instead of six (tiny transfers are
    # issue-bound at ~650ns each on the shared HWDGE device):
    #   cols 0:16  = ow  [128 h-part, (t, 2)]
    #   col  16    = sv  (partitions 0:16)
    #   cols 17:25 = sel as bf16 bitcast (partitions 0:16)
    #   row 0, cols 25:33 = ones; cols 33:35 = ob
    CB = 35
    cb = nc.dram_tensor("cb", [128, CB], F32, kind="ExternalInput")
    out = nc.dram_tensor("out", [B_LOC, 2], F32, kind="ExternalOutput")

    NG = (C + G - 1) // G

    with tile.TileContext(nc) as tc:
        with (
            tc.tile_pool(name="consts", bufs=1) as consts,
            tc.tile_pool(name="embp", bufs=8) as embp,
            tc.tile_pool(name="small", bufs=1) as small,
            tc.tile_pool(name="acc", bufs=1, space="PSUM") as accp,
            tc.tile_pool(name="pxt", bufs=1, space="PSUM") as pxtp,
            tc.tile_pool(name="ph", bufs=1, space="PSUM") as php,
            tc.tile_pool(name="pl", bufs=1, space="PSUM") as plp,
        ):
            # the mask tile and the first embedding group go first so the
            # DMA engines saturate immediately; small consts follow.
            wm_t = consts.tile([128, C * 16], F8, tag="wm")
            nc.sync.dma_start(out=wm_t[:], in_=wm.ap())

            dw_t = consts.tile([128, KD, D], BF16, tag="dw")
            db_t = consts.tile([1, D], F32, tag="db")
            cb_t = consts.tile([128, CB], F32, tag="cb")
            sv_t = cb_t[0:16, 16:17]
            sel_t = cb_t[0:16, 17:25].bitcast(BF16)
            ones_t = cb_t[0:1, 25 : 25 + B_LOC]
            ob_t = cb_t[0:1, 33:35]

            # warm the ScalarE Tanh LUT during the embedding stream so the
            # serial tail doesn't pay the ACT table load
            warm = consts.tile([1, 8], F32, tag="warm")
            nc.vector.memset(warm[:], 0.0)
            nc.scalar.activation(warm[:], warm[:], mybir.ActivationFunctionType.Tanh)

            # ---- phase 1: masked segment sums ----------------------------
            # px2[h][q, d'] accumulates sum_r wm[r, q] * embs[r, h*512+d']
            # over every packed row r; q = 2*slot + sent routes rows.
            px2 = [accp.tile([16, 512], F32, name=f"px2_{h}", tag=f"px2_{h}") for h in range(2)]
            pht = php.tile([128, KH, B_LOC], F32, tag="pht")
            pl = plp.tile([B_LOC, 2], F32, tag="pl")

            dwi = 0
            for g in range(NG):
                w = min(G, C - g * G)
                et = embp.tile([128, G, D], F8, tag="emb")
                src = embs.ap()[g * G * 128 : (g * G + w) * 128, :]
                nc.sync.dma_start(
                    out=et[:, :w, :], in_=src.rearrange("(n p) d -> p n d", p=128)
                )
                if g == 0:
                    nc.sync.dma_start(out=cb_t[:], in_=cb.ap())
                elif g == 1:
                    nc.sync.dma_start(out=db_t[:], in_=db.ap())
                # spread the 16 dense_w chunk loads evenly through the stream
                while dwi < KD and dwi * NG <= g * KD:
                    nc.sync.dma_start(
                        out=dw_t[:, dwi, :],
                        in_=dw.ap()[dwi * 128 : (dwi + 1) * 128, :],
                    )
                    dwi += 1
                if g == min(2, NG - 1):
                    # bias-init matmuls: first write (start=True) of the
                    # dense/logits PSUM regions, off the critical tail path
                    for t in range(KH):
                        nc.tensor.matmul(
                            pht[:, t, :],
                            db_t[0:1, t * 128 : (t + 1) * 128],
                            ones_t,
                            start=True,
                            stop=False,
                        )
                    nc.tensor.matmul(pl[:], ones_t, ob_t, start=True, stop=False)
                for k in range(w):
                    c = g * G + k
                    for h in range(2):
                        nc.tensor.matmul(
                            px2[h][:],
                            wm_t[:, c * 16 : c * 16 + 16],
                            et[:, k, h * 512 : h * 512 + 512],
                            start=(c == 0),
                            stop=(c == C - 1),
                        )
            while dwi < KD:
                nc.sync.dma_start(
                    out=dw_t[:, dwi, :], in_=dw.ap()[dwi * 128 : (dwi + 1) * 128, :]
                )
                dwi += 1

            # ---- exact 1/n scale (DVE and ACT halves run concurrently) ---
            x2s = small.tile([16, D], BF16, tag="x2s")
            nc.vector.tensor_scalar_mul(x2s[:, 0:512], px2[0][:], sv_t)
            nc.scalar.mul(x2s[:, 512:1024], px2[1][:], sv_t)

            # ---- transpose x2s [16, 2048-feature] -> xt [128, (r t j)] ---
            ptx = pxtp.tile([128, KH, 2, B_LOC], F32, tag="ptx")
            for t in range(KH):
                nc.tensor.matmul(
                    ptx[:, t, :, :],
                    x2s[:, t * 128 : t * 128 + 128],
                    sel_t,
                    start=True,
                    stop=True,
                )
            xt = small.tile([128, KD * B_LOC], BF16, tag="xt")
            xt4 = xt[:].rearrange("p (r t j) -> p t r j", r=2, t=KH)
            nc.vector.tensor_copy(xt4, ptx[:])

            # ---- phase 2: hidden^T = dense_w^T x + db (PSUM [h, j]) ------
            for kk in range(KD):
                for t in range(KH):
                    nc.tensor.matmul(
                        pht[:, t, :],
                        dw_t[:, kk, t * 128 : (t + 1) * 128],
                        xt[:, kk * B_LOC : (kk + 1) * B_LOC],
                        start=False,
                        stop=(kk == KD - 1),
                    )
            ht = small.tile([128, KH * B_LOC], F32, tag="ht")
            nc.scalar.activation(
                ht[:],
                pht[:].rearrange("p k j -> p (k j)"),
                mybir.ActivationFunctionType.Tanh,
            )

            # ---- phase 3: logits = hidden @ out_w + ob -------------------
            for t in range(KH):
                nc.tensor.matmul(
                    pl[:],
                    ht[:, t * B_LOC : (t + 1) * B_LOC],
                    cb_t[:, 2 * t : 2 * t + 2],
                    start=False,
                    stop=(t == KH - 1),
                )
            lg = small.tile([B_LOC, 2], F32, tag="lg")
            nc.vector.tensor_copy(lg[:], pl[:])
            nc.sync.dma_start(out=out.ap(), in_=lg[:])

    _split_excess_waits(nc)
    return nc


def _balance(rows):
    """Assign 64 samples to 8 cores (8 each), minimizing the max total rows.
    LPT greedy then swap refinement."""
    order = np.argsort(-rows, kind="stable")
    assign = [[] for _ in range(N_CORES)]
    loads = np.zeros(N_CORES, dtype=np.int64)
    for b in order:
        cands = np.argsort(loads, kind="stable")
        for c in cands:
            if len(assign[c]) < B_LOC:
                assign[c].append(int(b))
                loads[c] += rows[b]
                break
    # swap refinement: move the max down by pairwise swaps
    for _ in range(200):
        hi = int(np.argmax(loads))
        best = None
        for lo in range(N_CORES):
            if lo == hi:
                continue
            for i, bi in enumerate(assign[hi]):
                for j, bj in enumerate(assign[lo]):
                    d = rows[bi] - rows[bj]
                    if d <= 0:
                        continue
                    new_hi = loads[hi] - d
                    new_lo = loads[lo] + d
                    new_max = max(new_hi, new_lo)
                    if new_max < loads[hi] and (best is None or new_max < best[0]):
                        best = (new_max, lo, i, j)
        if best is None:
            break
        _, lo, i, j = best
        bi, bj = assign[hi][i], assign[lo][j]
        assign[hi][i], assign[lo][j] = bj, bi
        d = rows[bi] - rows[bj]
        loads[hi] -= d
        loads[lo] += d
    return assign, loads


_PROGRAM_CACHE = {}
LAST_RESULTS = None


def kernel(embs, input_ids, dense_w, dense_b, out_w, out_b):
    embs = np.asarray(embs, dtype=np.float32)
    ids = np.asarray(input_ids)
    dense_w = np.asarray(dense_w, dtype=np.float32)
    dense_b = np.asarray(dense_b, dtype=np.float32)
    out_w = np.asarray(out_w, dtype=np.float32)
    out_b = np.asarray(out_b, dtype=np.float32)

    # host-side segment metadata — exactly the reference's argmax semantics
    idx1 = np.argmax(ids == EOS_ID, axis=-1)
    idx2 = np.argmax(ids == PAD_ID, axis=-1) - 1
    # sent1 rows [1, idx1); sent2 rows [idx1+2, idx2)
    lo1 = np.minimum(1, np.maximum(idx1, 0))
    n1 = np.maximum(idx1 - 1, 0)
    lo2 = idx1 + 2
    n2 = np.maximum(idx2 - lo2, 0)
    nan_rows = (n1 == 0) | (n2 == 0)
    rows = n1 + n2

    assign, loads = _balance(rows)
    C = max(1, int(np.ceil(loads.max() / 128)))

    if C not in _PROGRAM_CACHE:
        _PROGRAM_CACHE[C] = _build_program(C)
    nc = _PROGRAM_CACHE[C]

    # selector: route q = 2j+r -> column r*8 + j
    sel_np = np.zeros((16, 16), np.float32)
    for r in range(2):
        for j in range(B_LOC):
            sel_np[2 * j + r, r * B_LOC + j] = 1.0

    dw8 = dense_w.astype(NP_BF16)
    db2 = dense_b.reshape(1, D).astype(np.float32)
    sel_bf = sel_np.astype(NP_BF16)

    # consts blob (layout documented in _build_program)
    cb_base = np.zeros((128, 35), np.float32)
    cb_base[:, 0:16] = out_w.reshape(8, 128, 2).transpose(1, 0, 2).reshape(128, 16)
    cb_base.view(np.uint16).reshape(128, 70)[0:16, 34:50] = sel_bf.view(np.uint16)
    cb_base[0, 25 : 25 + B_LOC] = 1.0
    cb_base[0, 33:35] = out_b.astype(np.float32)

    in_maps = []
    for core in range(N_CORES):
        samples = assign[core]
        packed = np.zeros((C * 128, D), np.float32)
        wmat = np.zeros((C * 128, 16), NP_F8)
        cb_np = cb_base.copy()
        off = 0
        for j, b in enumerate(samples):
            for r, (lo, n) in enumerate(((1, n1[b]), (lo2[b], n2[b]))):
                if n > 0:
                    packed[off : off + n] = embs[b, lo : lo + n]
                    wmat[off : off + n, 2 * j + r] = 1.0
                    cb_np[2 * j + r, 16] = np.float32(1.0) / np.float32(n)
                    off += n
        packed8 = packed.astype(NP_F8)
        # wm dram layout [128, C*16]: [p, c*16+q] = weight of row c*128+p
        wm_np = np.ascontiguousarray(
            wmat.reshape(C, 128, 16).transpose(1, 0, 2).reshape(128, C * 16)
        )
        in_maps.append(
            {
                "embs": packed8,
                "wm": wm_np,
                "dw": dw8,
                "db": db2,
                "cb": cb_np,
            }
        )

    res = run_bass_kernel_spmd(nc, in_maps, list(range(N_CORES)))
    global LAST_RESULTS
    LAST_RESULTS = res

    logits = np.empty((B, 2), np.float32)
    for core in range(N_CORES):
        logits[assign[core]] = res.results[core]["out"]
    logits[nan_rows] = np.nan
    return logits
